# revision 16
# baseline (speedup 1.0000x reference)
"""Bass/Trainium2 kernel for nn_KernelAMController (retrieval_knn).

Math: out(b,:) = -sum_g w_eff(b,g)*adj[tb(b),g,:] / (sum_g w_eff(b,g) + eps)
with w_eff(b,g) = exp(-2*||x_b - p_g||^2) * (counts[tb(b),g] > 0).

Key optimization over the dense version: the Gaussian (bandwidth 0.5) has
support radius ~1.8, and each 128-point grid chunk spans only ~0.84 in x.
The host sorts queries by x-coordinate, so each 512-sample group needs at
most 6 of the 20 grid chunks (selected host-side, gathered into per-group
operand buffers; short groups are padded with a dummy chunk whose exponent
column is -1e30 and whose adjoint block is zero).

Per group (512 samples, data-parallel over B on 8 cores):
  mm1: W^T(g,b) = exp(Pa^T @ Xa) for the 6 selected chunks, K=15 split-bf16
       augmented matmul. Chunks run 3-at-a-time CONCURRENTLY in the PE
       array via row tiling (K=15 slots at partition bases 0/32/64/96),
       into one 3-bank PSUM tile so exp is 2 ACTIVATEs of N=1536.
  mm2: Y^T(m,b) += Ct(g,m) * W^T(g,b); even slots accumulate into PSUM
       partitions 0:64, odd slots into 64:128 (col tiling, concurrent).
  One-hot bin-select mask comes precomputed from the host (o3full);
  applied to Y^T, reduced over bins with a +/-1 block-matrix matmul
  (negation folded in). The [3,BG] result (num_x, num_y, den) is copied
  to SBUF and DMA'd out; the host does the final divide/transpose/unsort.
"""
import numpy as np
import ml_dtypes

import concourse.bass as bass
import concourse.tile as tile
from concourse import mybir, bacc
from concourse.bass_utils import run_bass_kernel_spmd

F32 = mybir.dt.float32
BF16 = mybir.dt.bfloat16
BF16_NP = ml_dtypes.bfloat16

B = 32768
G = 2500
GP = 2560          # padded grid (20 chunks of 128)
NCHUNK = 20
NBINS = 20
NCORES = 8
BC = B // NCORES   # 4096 samples per core
NGRP = 8           # groups per core
BG = BC // NGRP    # 512 samples per group
NSEL = 6           # grid chunks retained per group (2 triples)
RADIUS = 1.8       # x-distance truncation for chunk selection
EPS = 1e-10

_CACHE = {}


def _build_nc():
    nc = bacc.Bacc("TRN2", target_bir_lowering=False)
    xa4_d = nc.dram_tensor("xa4", [128, BC], BF16, kind="ExternalInput")
    o3_d = nc.dram_tensor("o3full", [128, BC], BF16, kind="ExternalInput")
    pa4_d = nc.dram_tensor("pa4", [128, NGRP * 2 * 128], BF16,
                           kind="ExternalInput")
    ct4_d = nc.dram_tensor("ct4", [128, NGRP * NSEL * 64], BF16,
                           kind="ExternalInput")
    bn_d = nc.dram_tensor("bn128", [128, 3], BF16, kind="ExternalInput")
    o_d = nc.dram_tensor("o", [NGRP, 3, BG], F32, kind="ExternalOutput")

    with tile.TileContext(nc) as tc:
        with (
            tc.tile_pool(name="consts", bufs=1) as consts,
            tc.tile_pool(name="wt", bufs=4) as wtp,
            tc.tile_pool(name="r3", bufs=2) as r3p,
            tc.tile_pool(name="os", bufs=2) as osp,
            tc.tile_pool(name="pw", bufs=2, space="PSUM") as pwp,
            tc.tile_pool(name="py", bufs=1, space="PSUM") as pyp,
            tc.tile_pool(name="pr", bufs=1, space="PSUM") as prp,
        ):
            # warm the exp table load under the const DMAs
            dm = consts.tile([1, 1], F32)
            nc.vector.memset(dm[:], 0.0)
            dm2 = consts.tile([1, 1], F32)
            nc.scalar.activation(dm2[:], dm[:],
                                 mybir.ActivationFunctionType.Exp)

            pa4_sb = consts.tile([128, NGRP * 2 * 128], BF16)
            nc.sync.dma_start(out=pa4_sb[:], in_=pa4_d[:])
            xa4_sb = consts.tile([128, BC], BF16)
            nc.sync.dma_start(out=xa4_sb[:], in_=xa4_d[:])
            ct4_sb = consts.tile([128, NGRP * NSEL * 64], BF16)
            nc.sync.dma_start(out=ct4_sb[:], in_=ct4_d[:])
            o3_sb = consts.tile([128, BC], BF16)
            nc.sync.dma_start(out=o3_sb[:], in_=o3_d[:])
            bn_sb = consts.tile([128, 3], BF16)
            nc.sync.dma_start(out=bn_sb[:], in_=bn_d[:])

            # stage A(g): mm1 triples + exp -> wt tiles
            # stage B(g): mm2 + bin-select + reduce + copy out
            # Emission order A(0), A(1), B(0), A(2), B(1), ... keeps the
            # in-order PE queue fed with group g+1's mm1 while ScalarE is
            # still working, so exp never waits on group g's tail.
            def stage_a(g):
                gs = g * BG
                wts = []
                for tr in range(2):
                    pw = pwp.tile([128, 3, BG], F32)
                    for k in range(3):
                        s = 3 * tr + k
                        rb = 32 * (s % 4)
                        nc.tensor.matmul(
                            pw[:, k, :],
                            lhsT=pa4_sb[rb:rb + 15,
                                        (g * 2 + tr) * 128:
                                        (g * 2 + tr + 1) * 128],
                            rhs=xa4_sb[rb:rb + 15, gs:gs + BG],
                            start=True, stop=True, tile_position=(rb, 0))
                    wt = wtp.tile([128, 3, BG], BF16)
                    nc.scalar.activation(wt[:], pw[:],
                                         mybir.ActivationFunctionType.Exp)
                    wts.append(wt)
                return wts

            def stage_b(g, wts):
                gs = g * BG
                py = pyp.tile([128, BG], F32)
                for tr in range(2):
                    for k in range(3):
                        s = 3 * tr + k
                        out = py[0:64] if s % 2 == 0 else py[64:128]
                        nc.tensor.matmul(
                            out,
                            lhsT=ct4_sb[:, (g * NSEL + s) * 64:
                                        (g * NSEL + s + 1) * 64],
                            rhs=wts[tr][:, k, :], start=(s < 2),
                            stop=(s >= NSEL - 2), skip_group_check=True)
                # bin-select then reduce over bins (negation in bn128);
                # rows 60:64 / 124:128 of o3full are host-zeroed and the
                # matching py rows are exact zeros (ct pad columns)
                r3 = r3p.tile([128, BG], BF16)
                nc.vector.tensor_mul(r3[:], py[:], o3_sb[:, gs:gs + BG])
                pr = prp.tile([3, BG], F32)
                nc.tensor.matmul(pr[:], lhsT=bn_sb[:], rhs=r3[:],
                                 start=True, stop=True)
                osb = osp.tile([3, BG], F32)
                nc.vector.tensor_copy(osb[:], pr[:])
                nc.sync.dma_start(out=o_d[g], in_=osb[:])

            pend = None
            for g in range(NGRP):
                wts = stage_a(g)
                if pend is not None:
                    stage_b(*pend)
                pend = (g, wts)
            stage_b(*pend)
    nc.compile()
    return nc


def _host_prep(t, x, grid_points, grid_adjoints, t_edges, grid_counts):
    t = np.asarray(t, np.float32).reshape(B)
    x = np.asarray(x, np.float32)
    gp = np.asarray(grid_points, np.float32)
    adj = np.asarray(grid_adjoints, np.float32)
    te = np.asarray(t_edges, np.float32)
    cnt = np.asarray(grid_counts)

    order = np.argsort(x[:, 0], kind="stable")
    xs = x[order]
    ts = t[order]

    # grid chunk x-extents (points are x-major: idx = ix*50 + iy)
    gx = gp[:, 0]
    chunk_xmin = np.array([gx[128 * c] for c in range(NCHUNK)], np.float32)
    chunk_xmax = np.array([gx[min(128 * c + 127, G - 1)]
                           for c in range(NCHUNK)], np.float32)

    # grid operands, bf16 hi/lo split (pad columns G..GP-1 get -1e30)
    p5 = np.zeros((5, GP), np.float32)
    p5[0, :G] = 4.0 * gp[:, 0]
    p5[1, :G] = 4.0 * gp[:, 1]
    p5[2, :G] = -2.0
    p5[3, :G] = -2.0
    p5[4, :G] = -2.0 * (gp[:, 0] ** 2 + gp[:, 1] ** 2)
    p5[4, G:] = -1e30
    ph = p5.astype(BF16_NP)
    pl = (p5 - ph.astype(np.float32)).astype(BF16_NP)
    pa15 = np.concatenate([ph, ph, pl], axis=0)        # (15, GP) bf16
    pa_dummy = np.zeros((15, 128), BF16_NP)
    pa_dummy[4, :] = BF16_NP(-1e30)

    mask = (cnt > 0).astype(np.float32)                # (20, G)
    ct = np.zeros((GP, 64), np.float32)
    ct[:G, 0:20] = (mask * adj[:, :, 0]).T
    ct[:G, 20:40] = (mask * adj[:, :, 1]).T
    ct[:G, 40:60] = mask.T
    ct_chunks = ct.reshape(NCHUNK, 128, 64).astype(BF16_NP)
    ct_dummy = np.zeros((128, 64), BF16_NP)

    # x augmentation rows: [xh(2), sqh(2), 1, xl(2), sql(2), 0, xh(2),
    # sqh(2), 1] matching pa15 = [ph, ph, pl]; replicated at partition
    # bases 0/32/64/96 for PE row tiling
    xT = xs.T                                          # (2, B)
    sq = xT * xT
    xh = xT.astype(BF16_NP)
    xl = (xT - xh.astype(np.float32)).astype(BF16_NP)
    sqh = sq.astype(BF16_NP)
    sql = (sq - sqh.astype(np.float32)).astype(BF16_NP)
    xaug = np.zeros((15, B), BF16_NP)
    for base in (0, 5, 10):
        xaug[base + 0:base + 2] = xl if base == 5 else xh
        xaug[base + 2:base + 4] = sql if base == 5 else sqh
        xaug[base + 4] = BF16_NP(0.0 if base == 5 else 1.0)
    xa4 = np.zeros((128, B), BF16_NP)
    for rb in (0, 32, 64, 96):
        xa4[rb:rb + 15] = xaug

    # host-computed one-hot bin mask, d-major layout matching py rows
    # (rows d*20+k and 64+d*20+k; rows 60:64 and 124:128 stay zero)
    tb = np.clip(np.searchsorted(te[1:NBINS], ts, side="left"),
                 0, NBINS - 1)                         # (B,)
    oh = np.zeros((NBINS, B), BF16_NP)
    oh[tb, np.arange(B)] = BF16_NP(1.0)
    o3full = np.zeros((128, B), BF16_NP)
    for half in (0, 64):
        for dd in range(3):
            o3full[half + dd * 20:half + (dd + 1) * 20] = oh

    bn = np.zeros((128, 3), np.float32)
    for dd in range(3):
        v = 1.0 if dd == 2 else -1.0
        bn[dd * 20:(dd + 1) * 20, dd] = v
        bn[64 + dd * 20:64 + (dd + 1) * 20, dd] = v
    bn = bn.astype(BF16_NP)

    in_maps = []
    for i in range(NCORES):
        cs = slice(i * BC, (i + 1) * BC)
        pa4 = np.zeros((128, NGRP * 2 * 128), BF16_NP)
        ct4 = np.zeros((128, NGRP * NSEL * 64), BF16_NP)
        for g in range(NGRP):
            seg = xs[i * BC + g * BG:i * BC + (g + 1) * BG, 0]
            gmin, gmax = seg.min(), seg.max()
            d = np.maximum(np.maximum(chunk_xmin - gmax, gmin - chunk_xmax),
                           0.0)
            sel = np.argsort(d, kind="stable")[:NSEL]
            sel = np.sort(sel[d[sel] <= RADIUS])
            for s in range(NSEL):
                blk = pa_dummy if s >= len(sel) else \
                    pa15[:, sel[s] * 128:(sel[s] + 1) * 128]
                ctb = ct_dummy if s >= len(sel) else ct_chunks[sel[s]]
                tr, rb = s // 3, 32 * (s % 4)
                pa4[rb:rb + 15, (g * 2 + tr) * 128:
                    (g * 2 + tr + 1) * 128] = blk
                ct4[:, (g * NSEL + s) * 64:(g * NSEL + s + 1) * 64] = ctb

        in_maps.append({
            "xa4": np.ascontiguousarray(xa4[:, cs]),
            "o3full": np.ascontiguousarray(o3full[:, cs]),
            "pa4": pa4, "ct4": ct4, "bn128": bn,
        })
    return in_maps, order


def kernel(t, x, grid_points, grid_adjoints, t_edges, grid_counts,
           trace=False, tmpdir=None):
    if "nc" not in _CACHE:
        _CACHE["nc"] = _build_nc()
    nc = _CACHE["nc"]
    in_maps, order = _host_prep(t, x, grid_points, grid_adjoints, t_edges,
                                grid_counts)
    res = run_bass_kernel_spmd(nc, in_maps, core_ids=list(range(NCORES)),
                               trace=trace, tmpdir=tmpdir)
    _CACHE["last_result"] = res
    out_sorted = np.empty((B, 2), np.float32)
    for i in range(NCORES):
        raw = res.results[i]["o"].astype(np.float32)   # (NGRP, 3, BG)
        num = raw[:, 0:2, :]
        den = raw[:, 2, :] + EPS
        seg = (num / den[:, None, :]).transpose(0, 2, 1).reshape(BC, 2)
        out_sorted[i * BC:(i + 1) * BC] = seg
    out = np.empty((B, 2), np.float32)
    out[order] = out_sorted
    return out


# revision 20
# speedup vs baseline: 1.0476x; 1.0476x over previous
"""Bass/Trainium2 kernel for nn_KernelAMController (retrieval_knn).

Math: out(b,:) = -sum_g w_eff(b,g)*adj[tb(b),g,:] / (sum_g w_eff(b,g) + eps)
with w_eff(b,g) = exp(-2*||x_b - p_g||^2) * (counts[tb(b),g] > 0).

Strategy: the Gaussian (bandwidth 0.5) support radius is ~1.6, so after the
host sorts queries by x-coordinate each 512-sample group only interacts
with a narrow x-band of the grid. The grid is gathered host-side at
64-point granularity into per-group operand blocks of 128 points; a static
per-group-index slot profile (computed from the data, groups reordered
within each core to fit it) keeps the compiled program identical on all
cores while padding as little dummy work as possible.

Per group:
  mm1: W^T = exp(Pa^T @ Xa) over the selected blocks, K=15 split-bf16
       augmented matmul, 3 blocks CONCURRENT in the PE via row tiling
       (partition bases 0/32/64/96), exp in N=512*m ACTIVATE batches.
  mm2: Y^T(m,b) += Ct(g,m)*W^T(g,b); even slots -> PSUM 0:64, odd ->
       64:128 (col tiling, concurrent).
  Host-precomputed one-hot bin mask (o3full) * Y^T, reduced over bins by
  a +/-1 block matmul (output negation folded in). The [3,BG] result
  (num_x, num_y, den) goes back to the host, which divides/unsorts.
Inputs stream in per-group tiles spread across four engine DMA queues so
group 0's operands land ~100KB into the transfer instead of 3MB.
"""
import numpy as np
import ml_dtypes

import concourse.bass as bass
import concourse.tile as tile
from concourse import mybir, bacc
from concourse.bass_utils import run_bass_kernel_spmd

F32 = mybir.dt.float32
BF16 = mybir.dt.bfloat16
BF16_NP = ml_dtypes.bfloat16

B = 32768
G = 2500
GP = 2560          # padded grid (20 chunks of 128)
NGRAN = 40         # 64-point granules
NBINS = 20
NCORES = 8
BC = B // NCORES   # 4096 samples per core
NGRP = 8           # groups per core
BG = BC // NGRP    # 512 samples per group
MAXBLK = 6         # hard cap on 128-point blocks per group
RADIUS = 1.6       # x-distance truncation for granule selection
EPS = 1e-10

_CACHE = {}


def _batches(nblk):
    """Split a block count into mm1 batch sizes (3s then a 2/remainder)."""
    out = []
    while nblk >= 3:
        if nblk == 4:
            out += [2, 2]
            nblk = 0
            break
        out.append(3)
        nblk -= 3
    if nblk:
        out.append(nblk)
    return out


def _build_nc(profile):
    nblks = list(profile)
    tot_slots = sum(nblks)
    tot_batches = sum(len(_batches(n)) for n in nblks)

    nc = bacc.Bacc("TRN2", target_bir_lowering=False)
    xa4_d = nc.dram_tensor("xa4", [128, BC], BF16, kind="ExternalInput")
    o3_d = nc.dram_tensor("o3full", [128, BC], BF16, kind="ExternalInput")
    pa4_d = nc.dram_tensor("pa4", [128, tot_batches * 128], BF16,
                           kind="ExternalInput")
    ct4_d = nc.dram_tensor("ct4", [128, tot_slots * 64], BF16,
                           kind="ExternalInput")
    bn_d = nc.dram_tensor("bn128", [128, 3], BF16, kind="ExternalInput")
    o_d = nc.dram_tensor("o", [NGRP, 3, BG], F32, kind="ExternalOutput")

    with tile.TileContext(nc) as tc:
        with (
            tc.tile_pool(name="consts", bufs=1) as consts,
            tc.tile_pool(name="wt", bufs=4) as wtp,
            tc.tile_pool(name="r3", bufs=2) as r3p,
            tc.tile_pool(name="os", bufs=2) as osp,
            tc.tile_pool(name="pw", bufs=2, space="PSUM") as pwp,
            tc.tile_pool(name="py", bufs=1, space="PSUM") as pyp,
            tc.tile_pool(name="pr", bufs=1, space="PSUM") as prp,
        ):
            # warm the exp table load under the input DMAs
            dm = consts.tile([1, 1], F32)
            nc.vector.memset(dm[:], 0.0)
            dm2 = consts.tile([1, 1], F32)
            nc.scalar.activation(dm2[:], dm[:],
                                 mybir.ActivationFunctionType.Exp)

            bn_sb = consts.tile([128, 3], BF16)
            nc.scalar.dma_start(out=bn_sb[:], in_=bn_d[:])

            # per-group input tiles, streamed on four queues so group 0
            # can start as soon as its own slices land
            pa_t, ct_t, xa_t = [], [], []
            boff = soff = 0
            for g in range(NGRP):
                nb = len(_batches(nblks[g]))
                pa = consts.tile([128, nb * 128], BF16, tag=f"pa{g}")
                nc.sync.dma_start(
                    out=pa[:], in_=pa4_d[:, boff * 128:(boff + nb) * 128])
                ct = consts.tile([128, nblks[g] * 64], BF16, tag=f"ct{g}")
                nc.scalar.dma_start(
                    out=ct[:], in_=ct4_d[:, soff * 64:(soff + nblks[g]) * 64])
                xa = consts.tile([128, BG], BF16, tag=f"xa{g}")
                nc.gpsimd.dma_start(
                    out=xa[:], in_=xa4_d[:, g * BG:(g + 1) * BG])
                pa_t.append(pa)
                ct_t.append(ct)
                xa_t.append(xa)
                boff += nb
                soff += nblks[g]
            o3_t = []
            for h in range(2):
                o3 = consts.tile([128, BC // 2], BF16, tag=f"o3{h}")
                nc.gpsimd.dma_start(
                    out=o3[:], in_=o3_d[:, h * (BC // 2):(h + 1) * (BC // 2)])
                o3_t.append(o3)

            for g in range(NGRP):
                nblk = nblks[g]
                py = pyp.tile([128, BG], F32)
                s = 0
                wts = []        # (wt_tile, first_slot, m)
                for m in _batches(nblk):
                    pw = pwp.tile([128, 3, BG], F32)
                    for k in range(m):
                        rb = 32 * ((s + k) % 4)
                        nc.tensor.matmul(
                            pw[:, k, :],
                            lhsT=pa_t[g][rb:rb + 15,
                                         len(wts) * 128:(len(wts) + 1) * 128],
                            rhs=xa_t[g][rb:rb + 15, :],
                            start=True, stop=True, tile_position=(rb, 0))
                    wt = wtp.tile([128, 3, BG], BF16)
                    nc.scalar.activation(wt[:, 0:m, :], pw[:, 0:m, :],
                                         mybir.ActivationFunctionType.Exp)
                    # software pipeline: emit previous batch's mm2 while
                    # ScalarE computes this batch's exp
                    if wts:
                        pwt, ps, pm = wts[-1]
                        for k in range(pm):
                            sl = ps + k
                            out = py[0:64] if sl % 2 == 0 else py[64:128]
                            nc.tensor.matmul(
                                out, lhsT=ct_t[g][:, sl * 64:(sl + 1) * 64],
                                rhs=pwt[:, k, :], start=(sl < 2),
                                stop=(sl >= nblk - 2), skip_group_check=True)
                    wts.append((wt, s, m))
                    s += m
                pwt, ps, pm = wts[-1]
                for k in range(pm):
                    sl = ps + k
                    out = py[0:64] if sl % 2 == 0 else py[64:128]
                    nc.tensor.matmul(
                        out, lhsT=ct_t[g][:, sl * 64:(sl + 1) * 64],
                        rhs=pwt[:, k, :], start=(sl < 2),
                        stop=(sl >= nblk - 2), skip_group_check=True)

                # bin-select then reduce over bins (negation in bn128);
                # rows 60:64 / 124:128 of o3full are host-zeroed and the
                # matching py rows are exact zeros (ct pad columns)
                r3 = r3p.tile([128, BG], BF16)
                h, ho = (0, g * BG) if g < NGRP // 2 else \
                    (1, (g - NGRP // 2) * BG)
                nc.vector.tensor_mul(r3[:], py[:], o3_t[h][:, ho:ho + BG])
                pr = prp.tile([3, BG], F32)
                nc.tensor.matmul(pr[:], lhsT=bn_sb[:], rhs=r3[:],
                                 start=True, stop=True)
                osb = osp.tile([3, BG], F32)
                nc.vector.tensor_copy(osb[:], pr[:])
                nc.sync.dma_start(out=o_d[g], in_=osb[:])
    nc.compile()
    return nc


def _host_prep(t, x, grid_points, grid_adjoints, t_edges, grid_counts):
    t = np.asarray(t, np.float32).reshape(B)
    x = np.asarray(x, np.float32)
    gp = np.asarray(grid_points, np.float32)
    adj = np.asarray(grid_adjoints, np.float32)
    te = np.asarray(t_edges, np.float32)
    cnt = np.asarray(grid_counts)

    order = np.argsort(x[:, 0], kind="stable")

    # granule x-extents (points are x-major: idx = ix*50 + iy)
    gx = gp[:, 0]
    gran_xmin = np.array([gx[64 * u] for u in range(NGRAN)], np.float32)
    gran_xmax = np.array([gx[min(64 * u + 63, G - 1)]
                          for u in range(NGRAN)], np.float32)

    # grid operands, bf16 hi/lo split; granule NGRAN is an all-dummy pad
    # (exponent -1e30 -> w=0, adjoints 0)
    p5 = np.zeros((5, GP + 64), np.float32)
    p5[0, :G] = 4.0 * gp[:, 0]
    p5[1, :G] = 4.0 * gp[:, 1]
    p5[2, :G] = -2.0
    p5[3, :G] = -2.0
    p5[4, :G] = -2.0 * (gp[:, 0] ** 2 + gp[:, 1] ** 2)
    p5[4, G:] = -1e30
    ph = p5.astype(BF16_NP)
    pl = (p5 - ph.astype(np.float32)).astype(BF16_NP)
    pa15 = np.concatenate([ph, ph, pl], axis=0)        # (15, GP+64) bf16

    mask = (cnt > 0).astype(np.float32)                # (20, G)
    ct = np.zeros((GP + 64, 64), np.float32)
    ct[:G, 0:20] = (mask * adj[:, :, 0]).T
    ct[:G, 20:40] = (mask * adj[:, :, 1]).T
    ct[:G, 40:60] = mask.T
    ct64 = ct.reshape(NGRAN + 1, 64, 64).astype(BF16_NP)

    # per-group granule selection on the x-sorted data
    xs0 = x[order, 0]
    sels, nblks = [], []
    for gg in range(B // BG):
        seg = xs0[gg * BG:(gg + 1) * BG]
        a, b = seg.min(), seg.max()
        d = np.maximum(np.maximum(gran_xmin - b, a - gran_xmax), 0.0)
        near = np.argsort(d, kind="stable")[:2 * MAXBLK]
        sel = np.sort(near[d[near] <= RADIUS])
        sels.append(sel)
        nblks.append((len(sel) + 1) // 2)

    # reorder groups within each core (descending block count) and derive
    # the static per-index profile shared by all cores
    nblks = np.array(nblks).reshape(NCORES, NGRP)
    perm = np.argsort(-nblks, axis=1, kind="stable")   # (NCORES, NGRP)
    profile = tuple(max(int(v), 2) for v in
                    np.max(np.take_along_axis(nblks, perm, axis=1), axis=0))
    order = order.reshape(NCORES, NGRP, BG)
    order = np.take_along_axis(order, perm[:, :, None], axis=1).reshape(-1)

    ts = t[order]
    xs = x[order]

    # x augmentation rows: [xh(2), sqh(2), 1, xl(2), sql(2), 0, xh(2),
    # sqh(2), 1] matching pa15 = [ph, ph, pl]; replicated at partition
    # bases 0/32/64/96 for PE row tiling
    xT = xs.T
    sq = xT * xT
    xh = xT.astype(BF16_NP)
    xl = (xT - xh.astype(np.float32)).astype(BF16_NP)
    sqh = sq.astype(BF16_NP)
    sql = (sq - sqh.astype(np.float32)).astype(BF16_NP)
    xaug = np.zeros((15, B), BF16_NP)
    for base in (0, 5, 10):
        xaug[base + 0:base + 2] = xl if base == 5 else xh
        xaug[base + 2:base + 4] = sql if base == 5 else sqh
        xaug[base + 4] = BF16_NP(0.0 if base == 5 else 1.0)
    xa4 = np.zeros((128, B), BF16_NP)
    for rb in (0, 32, 64, 96):
        xa4[rb:rb + 15] = xaug

    # host-computed one-hot bin mask, d-major layout matching py rows
    tb = np.clip(np.searchsorted(te[1:NBINS], ts, side="left"),
                 0, NBINS - 1)
    oh = np.zeros((NBINS, B), BF16_NP)
    oh[tb, np.arange(B)] = BF16_NP(1.0)
    o3full = np.zeros((128, B), BF16_NP)
    for half in (0, 64):
        for dd in range(3):
            o3full[half + dd * 20:half + (dd + 1) * 20] = oh

    bn = np.zeros((128, 3), np.float32)
    for dd in range(3):
        v = 1.0 if dd == 2 else -1.0
        bn[dd * 20:(dd + 1) * 20, dd] = v
        bn[64 + dd * 20:64 + (dd + 1) * 20, dd] = v
    bn = bn.astype(BF16_NP)

    tot_slots = sum(profile)
    tot_batches = sum(len(_batches(n)) for n in profile)
    in_maps = []
    for i in range(NCORES):
        cs = slice(i * BC, (i + 1) * BC)
        pa4 = np.zeros((128, tot_batches * 128), BF16_NP)
        ct4 = np.zeros((128, tot_slots * 64), BF16_NP)
        boff = soff = 0
        for g in range(NGRP):
            gg_orig = i * NGRP + perm[i, g]
            sel = list(sels[gg_orig])
            nblk = profile[g]
            # granule pairs -> 128-point blocks, dummy-padded
            while len(sel) < 2 * nblk:
                sel.append(NGRAN)
            s = 0
            for bi, m in enumerate(_batches(nblk)):
                for k in range(m):
                    ua, ub = sel[2 * (s + k)], sel[2 * (s + k) + 1]
                    rb = 32 * ((s + k) % 4)
                    col = (boff + bi) * 128
                    pa4[rb:rb + 15, col:col + 64] = pa15[:, ua * 64:
                                                         (ua + 1) * 64]
                    pa4[rb:rb + 15, col + 64:col + 128] = \
                        pa15[:, ub * 64:(ub + 1) * 64]
                    ct4[0:64, (soff + s + k) * 64:(soff + s + k + 1) * 64] \
                        = ct64[ua]
                    ct4[64:128, (soff + s + k) * 64:(soff + s + k + 1) * 64] \
                        = ct64[ub]
                s += m
            boff += len(_batches(nblk))
            soff += nblk
        in_maps.append({
            "xa4": np.ascontiguousarray(xa4[:, cs]),
            "o3full": np.ascontiguousarray(o3full[:, cs]),
            "pa4": pa4, "ct4": ct4, "bn128": bn,
        })
    return in_maps, order, profile


def kernel(t, x, grid_points, grid_adjoints, t_edges, grid_counts,
           trace=False, tmpdir=None):
    in_maps, order, profile = _host_prep(
        t, x, grid_points, grid_adjoints, t_edges, grid_counts)
    key = ("nc", profile)
    if key not in _CACHE:
        _CACHE[key] = _build_nc(profile)
    nc = _CACHE[key]
    res = run_bass_kernel_spmd(nc, in_maps, core_ids=list(range(NCORES)),
                               trace=trace, tmpdir=tmpdir)
    _CACHE["last_result"] = res
    out_sorted = np.empty((B, 2), np.float32)
    for i in range(NCORES):
        raw = res.results[i]["o"].astype(np.float32)   # (NGRP, 3, BG)
        num = raw[:, 0:2, :]
        den = raw[:, 2, :] + EPS
        seg = (num / den[:, None, :]).transpose(0, 2, 1).reshape(BC, 2)
        out_sorted[i * BC:(i + 1) * BC] = seg
    out = np.empty((B, 2), np.float32)
    out[order] = out_sorted
    return out


# revision 24
# speedup vs baseline: 1.1277x; 1.0764x over previous
"""Bass/Trainium2 kernel for nn_KernelAMController (retrieval_knn).

Math: out(b,:) = -sum_g w_eff(b,g)*adj[tb(b),g,:] / (sum_g w_eff(b,g) + eps)
with w_eff(b,g) = exp(-2*||x_b - p_g||^2) * (counts[tb(b),g] > 0).

Strategy: the Gaussian (bandwidth 0.5) support radius is ~1.6, so after the
host sorts queries by x-coordinate each 512-sample group only interacts
with a narrow x-band of the grid. The grid is gathered host-side at
64-point granularity into per-group operand blocks of 128 points; a static
per-group-index slot profile (computed from the data, groups reordered
within each core to fit it) keeps the compiled program identical on all
cores while padding as little dummy work as possible.

Per group:
  mm1: W^T = exp(Pa^T @ Xa) over the selected blocks, K=15 split-bf16
       augmented matmul, 3 blocks CONCURRENT in the PE via row tiling
       (partition bases 0/32/64/96), exp in N=512*m ACTIVATE batches.
  mm2: Y^T(m,b) += Ct(g,m)*W^T(g,b); even slots -> PSUM 0:64, odd ->
       64:128 (col tiling, concurrent).
  Host-precomputed one-hot bin mask (o3full) * Y^T, reduced over bins by
  a +/-1 block matmul (output negation folded in). The [3,BG] result
  (num_x, num_y, den) goes back to the host, which divides/unsorts.
Inputs stream in per-group tiles spread across four engine DMA queues so
group 0's operands land ~100KB into the transfer instead of 3MB.
"""
import numpy as np
import ml_dtypes

import concourse.bass as bass
import concourse.tile as tile
from concourse import mybir, bacc
from concourse.bass_utils import run_bass_kernel_spmd

F32 = mybir.dt.float32
BF16 = mybir.dt.bfloat16
BF16_NP = ml_dtypes.bfloat16

B = 32768
G = 2500
GP = 2560          # padded grid (20 chunks of 128)
NGRAN = 40         # 64-point granules
NBINS = 20
NCORES = 8
BC = B // NCORES   # 4096 samples per core
NGRP = 8           # groups per core
BG = BC // NGRP    # 512 samples per group
MAXBLK = 6         # hard cap on 128-point blocks per group
RADIUS = 1.6       # x-distance truncation for granule selection
EPS = 1e-10

_CACHE = {}


def _batches(nblk):
    """Split a block count into mm1 batch sizes (3s then a 2/remainder)."""
    out = []
    while nblk >= 3:
        if nblk == 4:
            out += [2, 2]
            nblk = 0
            break
        out.append(3)
        nblk -= 3
    if nblk:
        out.append(nblk)
    return out


def _build_nc(profile):
    nblks = list(profile)
    tot_slots = sum(nblks)
    tot_batches = sum(len(_batches(n)) for n in nblks)

    nc = bacc.Bacc("TRN2", target_bir_lowering=False)
    # per-group widths of the grid-side gather buffer (pa batches + ct)
    gws = [len(_batches(n)) * 128 + n * 64 for n in nblks]
    gin_d = nc.dram_tensor("gin", [128, sum(gws)], BF16,
                           kind="ExternalInput")
    xo_d = nc.dram_tensor("xo", [128, NGRP * 2 * BG], BF16,
                          kind="ExternalInput")
    bn_d = nc.dram_tensor("bn128", [128, 3], BF16, kind="ExternalInput")
    o_d = nc.dram_tensor("o", [NGRP, 3, BG], F32, kind="ExternalOutput")

    with tile.TileContext(nc) as tc:
        with (
            tc.tile_pool(name="consts", bufs=1) as consts,
            tc.tile_pool(name="wt", bufs=4) as wtp,
            tc.tile_pool(name="r3", bufs=2) as r3p,
            tc.tile_pool(name="os", bufs=2) as osp,
            tc.tile_pool(name="pw", bufs=2, space="PSUM") as pwp,
            tc.tile_pool(name="py", bufs=1, space="PSUM") as pyp,
            tc.tile_pool(name="pr", bufs=1, space="PSUM") as prp,
        ):
            # warm the exp table load under the input DMAs
            dm = consts.tile([1, 1], F32)
            nc.vector.memset(dm[:], 0.0)
            dm2 = consts.tile([1, 1], F32)
            nc.scalar.activation(dm2[:], dm[:],
                                 mybir.ActivationFunctionType.Exp)

            bn_sb = consts.tile([128, 3], BF16)
            nc.gpsimd.dma_start(out=bn_sb[:], in_=bn_d[:])

            # per-group input tiles on two queues (never ScalarE's — exp
            # must not sit behind DMA issues in its FIFO): grid-side
            # gather (pa|ct) on sync, sample-side (xa|o3) on gpsimd
            gin_t, xo_t = [], []
            goff = 0
            for g in range(NGRP):
                gin = consts.tile([128, gws[g]], BF16, tag=f"gin{g}")
                nc.sync.dma_start(out=gin[:],
                                  in_=gin_d[:, goff:goff + gws[g]])
                xo = consts.tile([128, 2 * BG], BF16, tag=f"xo{g}")
                nc.gpsimd.dma_start(
                    out=xo[:], in_=xo_d[:, g * 2 * BG:(g + 1) * 2 * BG])
                gin_t.append(gin)
                xo_t.append(xo)
                goff += gws[g]

            for g in range(NGRP):
                nblk = nblks[g]
                ctoff = len(_batches(nblk)) * 128
                py = pyp.tile([128, BG], F32)
                s = 0
                wts = []        # (wt_tile, first_slot, m)

                def mm2_batch(wt_, ps, pm):
                    for k in range(pm):
                        sl = ps + k
                        out = py[0:64] if sl % 2 == 0 else py[64:128]
                        nc.tensor.matmul(
                            out,
                            lhsT=gin_t[g][:, ctoff + sl * 64:
                                          ctoff + (sl + 1) * 64],
                            rhs=wt_[:, k, :], start=(sl < 2),
                            stop=(sl >= nblk - 2), skip_group_check=True)

                for m in _batches(nblk):
                    pw = pwp.tile([128, 3, BG], F32)
                    for k in range(m):
                        rb = 32 * ((s + k) % 4)
                        nc.tensor.matmul(
                            pw[:, k, :],
                            lhsT=gin_t[g][rb:rb + 15,
                                          len(wts) * 128:
                                          (len(wts) + 1) * 128],
                            rhs=xo_t[g][rb:rb + 15, 0:BG],
                            start=True, stop=True, tile_position=(rb, 0))
                    wt = wtp.tile([128, 3, BG], BF16)
                    nc.scalar.activation(wt[:, 0:m, :], pw[:, 0:m, :],
                                         mybir.ActivationFunctionType.Exp)
                    # software pipeline: emit previous batch's mm2 while
                    # ScalarE computes this batch's exp
                    if wts:
                        mm2_batch(*wts[-1])
                    wts.append((wt, s, m))
                    s += m
                mm2_batch(*wts[-1])

                # bin-select then reduce over bins (negation in bn128);
                # rows 60:64 / 124:128 of o3full are host-zeroed and the
                # matching py rows are exact zeros (ct pad columns)
                r3 = r3p.tile([128, BG], BF16)
                nc.vector.tensor_mul(r3[:], py[:], xo_t[g][:, BG:2 * BG])
                pr = prp.tile([3, BG], F32)
                nc.tensor.matmul(pr[:], lhsT=bn_sb[:], rhs=r3[:],
                                 start=True, stop=True)
                osb = osp.tile([3, BG], F32)
                nc.vector.tensor_copy(osb[:], pr[:])
                nc.sync.dma_start(out=o_d[g], in_=osb[:])
    nc.compile()
    return nc


def _host_prep(t, x, grid_points, grid_adjoints, t_edges, grid_counts):
    t = np.asarray(t, np.float32).reshape(B)
    x = np.asarray(x, np.float32)
    gp = np.asarray(grid_points, np.float32)
    adj = np.asarray(grid_adjoints, np.float32)
    te = np.asarray(t_edges, np.float32)
    cnt = np.asarray(grid_counts)

    order = np.argsort(x[:, 0], kind="stable")

    # granule x-extents (points are x-major: idx = ix*50 + iy)
    gx = gp[:, 0]
    gran_xmin = np.array([gx[64 * u] for u in range(NGRAN)], np.float32)
    gran_xmax = np.array([gx[min(64 * u + 63, G - 1)]
                          for u in range(NGRAN)], np.float32)

    # grid operands, bf16 hi/lo split; granule NGRAN is an all-dummy pad
    # (exponent -1e30 -> w=0, adjoints 0)
    p5 = np.zeros((5, GP + 64), np.float32)
    p5[0, :G] = 4.0 * gp[:, 0]
    p5[1, :G] = 4.0 * gp[:, 1]
    p5[2, :G] = -2.0
    p5[3, :G] = -2.0
    p5[4, :G] = -2.0 * (gp[:, 0] ** 2 + gp[:, 1] ** 2)
    p5[4, G:] = -1e30
    ph = p5.astype(BF16_NP)
    pl = (p5 - ph.astype(np.float32)).astype(BF16_NP)
    pa15 = np.concatenate([ph, ph, pl], axis=0)        # (15, GP+64) bf16

    mask = (cnt > 0).astype(np.float32)                # (20, G)
    ct = np.zeros((GP + 64, 64), np.float32)
    ct[:G, 0:20] = (mask * adj[:, :, 0]).T
    ct[:G, 20:40] = (mask * adj[:, :, 1]).T
    ct[:G, 40:60] = mask.T
    ct64 = ct.reshape(NGRAN + 1, 64, 64).astype(BF16_NP)

    # per-group granule selection on the x-sorted data
    xs0 = x[order, 0]
    sels, nblks = [], []
    for gg in range(B // BG):
        seg = xs0[gg * BG:(gg + 1) * BG]
        a, b = seg.min(), seg.max()
        d = np.maximum(np.maximum(gran_xmin - b, a - gran_xmax), 0.0)
        near = np.argsort(d, kind="stable")[:2 * MAXBLK]
        sel = np.sort(near[d[near] <= RADIUS])
        sels.append(sel)
        nblks.append((len(sel) + 1) // 2)

    # reorder groups within each core (descending block count) and derive
    # the static per-index profile shared by all cores
    nblks = np.array(nblks).reshape(NCORES, NGRP)
    perm = np.argsort(-nblks, axis=1, kind="stable")   # (NCORES, NGRP)
    profile = tuple(max(int(v), 2) for v in
                    np.max(np.take_along_axis(nblks, perm, axis=1), axis=0))
    order = order.reshape(NCORES, NGRP, BG)
    order = np.take_along_axis(order, perm[:, :, None], axis=1).reshape(-1)

    ts = t[order]
    xs = x[order]

    # x augmentation rows: [xh(2), sqh(2), 1, xl(2), sql(2), 0, xh(2),
    # sqh(2), 1] matching pa15 = [ph, ph, pl]; replicated at partition
    # bases 0/32/64/96 for PE row tiling
    xT = xs.T
    sq = xT * xT
    xh = xT.astype(BF16_NP)
    xl = (xT - xh.astype(np.float32)).astype(BF16_NP)
    sqh = sq.astype(BF16_NP)
    sql = (sq - sqh.astype(np.float32)).astype(BF16_NP)
    xaug = np.zeros((15, B), BF16_NP)
    for base in (0, 5, 10):
        xaug[base + 0:base + 2] = xl if base == 5 else xh
        xaug[base + 2:base + 4] = sql if base == 5 else sqh
        xaug[base + 4] = BF16_NP(0.0 if base == 5 else 1.0)
    xa4 = np.zeros((128, B), BF16_NP)
    for rb in (0, 32, 64, 96):
        xa4[rb:rb + 15] = xaug

    # host-computed one-hot bin mask, d-major layout matching py rows
    tb = np.clip(np.searchsorted(te[1:NBINS], ts, side="left"),
                 0, NBINS - 1)
    oh = np.zeros((NBINS, B), BF16_NP)
    oh[tb, np.arange(B)] = BF16_NP(1.0)
    o3full = np.zeros((128, B), BF16_NP)
    for half in (0, 64):
        for dd in range(3):
            o3full[half + dd * 20:half + (dd + 1) * 20] = oh

    bn = np.zeros((128, 3), np.float32)
    for dd in range(3):
        v = 1.0 if dd == 2 else -1.0
        bn[dd * 20:(dd + 1) * 20, dd] = v
        bn[64 + dd * 20:64 + (dd + 1) * 20, dd] = v
    bn = bn.astype(BF16_NP)

    # per-group widths of the grid-side gather buffer (pa batches + ct)
    gws = [len(_batches(n)) * 128 + n * 64 for n in profile]
    in_maps = []
    for i in range(NCORES):
        gin = np.zeros((128, sum(gws)), BF16_NP)
        xo = np.zeros((128, NGRP * 2 * BG), BF16_NP)
        goff = 0
        for g in range(NGRP):
            gg_orig = i * NGRP + perm[i, g]
            sel = list(sels[gg_orig])
            nblk = profile[g]
            ctoff = goff + len(_batches(nblk)) * 128
            # granule pairs -> 128-point blocks, dummy-padded
            while len(sel) < 2 * nblk:
                sel.append(NGRAN)
            s = 0
            for bi, m in enumerate(_batches(nblk)):
                for k in range(m):
                    ua, ub = sel[2 * (s + k)], sel[2 * (s + k) + 1]
                    rb = 32 * ((s + k) % 4)
                    col = goff + bi * 128
                    gin[rb:rb + 15, col:col + 64] = pa15[:, ua * 64:
                                                         (ua + 1) * 64]
                    gin[rb:rb + 15, col + 64:col + 128] = \
                        pa15[:, ub * 64:(ub + 1) * 64]
                    gin[0:64, ctoff + (s + k) * 64:ctoff + (s + k + 1) * 64] \
                        = ct64[ua]
                    gin[64:128, ctoff + (s + k) * 64:
                        ctoff + (s + k + 1) * 64] = ct64[ub]
                s += m
            cols = slice((i * NGRP + g) * BG, (i * NGRP + g + 1) * BG)
            xo[:, g * 2 * BG:g * 2 * BG + BG] = xa4[:, cols]
            xo[:, g * 2 * BG + BG:(g + 1) * 2 * BG] = o3full[:, cols]
            goff += gws[g]
        in_maps.append({"gin": gin, "xo": xo, "bn128": bn})
    return in_maps, order, profile


def kernel(t, x, grid_points, grid_adjoints, t_edges, grid_counts,
           trace=False, tmpdir=None):
    in_maps, order, profile = _host_prep(
        t, x, grid_points, grid_adjoints, t_edges, grid_counts)
    key = ("nc", profile)
    if key not in _CACHE:
        _CACHE[key] = _build_nc(profile)
    nc = _CACHE[key]
    res = run_bass_kernel_spmd(nc, in_maps, core_ids=list(range(NCORES)),
                               trace=trace, tmpdir=tmpdir)
    _CACHE["last_result"] = res
    out_sorted = np.empty((B, 2), np.float32)
    for i in range(NCORES):
        raw = res.results[i]["o"].astype(np.float32)   # (NGRP, 3, BG)
        num = raw[:, 0:2, :]
        den = raw[:, 2, :] + EPS
        seg = (num / den[:, None, :]).transpose(0, 2, 1).reshape(BC, 2)
        out_sorted[i * BC:(i + 1) * BC] = seg
    out = np.empty((B, 2), np.float32)
    out[order] = out_sorted
    return out
